# revision 1
# baseline (speedup 1.0000x reference)
"""Trainium2 Bass kernel for nn_Discriminator (stacked GRU discriminator).

Strategy: all heavy compute (six 256-step GRU chains, B=64, H=1024) runs on
one NeuronCore as a single fused Bass/Tile program; consecutive chains are
connected by DRAM tapes and the Tile scheduler pipelines independent chains
automatically. Matmuls run bf16 with fp32 PSUM accumulation, N-split across
the two PE column groups (interleaved issue), biases injected via K=1
ones-matmuls, h.T maintained via PE-mode transposes. Host does the embedding
gather, layout packing, and the tiny final score layer.
"""
import sys
sys.path.insert(0, "/opt/trn_rl_repo")
import time
import numpy as np
import ml_dtypes
import jax
from jax.sharding import Mesh, PartitionSpec
from jax.experimental.shard_map import shard_map
from concourse import bass, bacc, tile, mybir
from concourse.bass2jax import _bass_exec_p, install_neuronx_cc_hook, partition_id_tensor

BF16 = mybir.dt.bfloat16
F32 = mybir.dt.float32
H = 1024
KC = 8
B = 64
SIG = mybir.ActivationFunctionType.Sigmoid
TANH = mybir.ActivationFunctionType.Tanh
CPY = mybir.ActivationFunctionType.Copy


def emit_step(nc, sbuf, psum_rz, psum_n, psum_tr, W, x_t, t_hT, t_hf,
              out_nat_t=None, out_hT_t=None):
    """One GRU step, B=64 H=1024. x_t: AP [128, 512] bf16 (xT chunk layout)."""
    whhT, wihT = W["whhT"], W["wihT"]
    hT = lambda k: t_hT[k][:]
    xc = lambda k: x_t[:, k*64:(k+1)*64]

    p_rg = [psum_rz.tile([128, 512], F32, tag="prg", name=f"prg{c}") for c in range(2)]
    p_hz = [psum_n.tile([128, 512], F32, tag="phz", name=f"phz{c}") for c in range(2)]

    def fused(lst, ps, lo, bias, brow):
        lst.append((ps, W["ones"][:], bias[:, brow*512:(brow+1)*512], True, False))
        for k in range(KC):
            lst.append((ps, hT(k), whhT[:, k*3072+lo:k*3072+lo+512], False, False))
        for k in range(KC):
            lst.append((ps, xc(k), wihT[:, k*3072+lo:k*3072+lo+512], False, k == KC-1))

    def single(lst, ps, lo, bias, brow, use_x):
        lst.append((ps, W["ones"][:], bias[:, brow*512:(brow+1)*512], True, False))
        for k in range(KC):
            src = xc(k) if use_x else hT(k)
            wt = wihT if use_x else whhT
            lst.append((ps, src, wt[:, k*3072+lo:k*3072+lo+512], False, k == KC-1))

    A, Bq = [], []
    # A: r0(17), hn0(9), r1(17), hn1(9)
    fused(A, p_rg[0][0:64, :], 0, W["brz"], 0)
    single(A, p_hz[0][0:64, :], 2048, W["bhhn"], 0, False)
    fused(A, p_rg[1][0:64, :], 512, W["brz"], 1)
    single(A, p_hz[1][0:64, :], 2560, W["bhhn"], 1, False)
    # B: gin0(9), z2(17), gin1(9), z3(17)
    single(Bq, p_rg[0][64:128, :], 2048, W["bihn"], 0, True)
    fused(Bq, p_hz[0][64:128, :], 1024, W["brz"], 2)
    single(Bq, p_rg[1][64:128, :], 2560, W["bihn"], 1, True)
    fused(Bq, p_hz[1][64:128, :], 1536, W["brz"], 3)
    for i in range(max(len(A), len(Bq))):
        if i < len(A):
            oa, la, ra, sa, pa = A[i]
            nc.tensor.matmul(oa, la, ra, start=sa, stop=pa)
        if i < len(Bq):
            ob, lb, rb, sb, pb = Bq[i]
            nc.tensor.matmul(ob, lb, rb, start=sb, stop=pb, skip_group_check=True)

    # --- gates + state update, per half c ---
    bf16_state = t_hf[0].dtype == BF16 if hasattr(t_hf[0], "dtype") else False
    for c in range(2):
        r = sbuf.tile([64, 512], BF16, tag="rg", name="rg")
        nc.scalar.activation(r[:], p_rg[c][0:64, :], SIG)
        gin = sbuf.tile([64, 512], BF16, tag="gin", name="gin")
        nc.scalar.activation(gin[:], p_rg[c][64:128, :], CPY)      # cross-base move
        z = sbuf.tile([64, 512], BF16, tag="zg", name="zg")
        nc.scalar.activation(z[:], p_hz[c][64:128, :], SIG)        # cross-base move
        rhn = sbuf.tile([64, 512], F32, tag="rhn", name="rhn")
        nc.vector.tensor_mul(rhn[:], r[:], p_hz[c][0:64, :])
        npre = sbuf.tile([64, 512], F32, tag="npre", name="npre")
        nc.vector.tensor_add(npre[:], rhn[:], gin[:])
        nf = sbuf.tile([64, 512], F32, tag="nf", name="nf")
        nc.scalar.activation(nf[:], npre[:], TANH)
        hslice = t_hf[c][:]
        hmn = sbuf.tile([64, 512], F32, tag="hmn", name="hmn")
        nc.vector.tensor_sub(hmn[:], hslice, nf[:])
        zh = sbuf.tile([64, 512], F32, tag="zh", name="zh")
        nc.vector.tensor_mul(zh[:], z[:], hmn[:])
        nc.vector.tensor_add(hslice, nf[:], zh[:])
        hb = sbuf.tile([64, 512], BF16, tag="hb", name="hb")
        nc.vector.tensor_copy(hb[:], hslice)
        for j in range(4):
            kk = 4*c + j
            ptr = psum_tr.tile([128, 64], BF16, tag="tr", name="tr")
            nc.tensor.transpose(ptr[:], hb[:, j*128:(j+1)*128], W["eye"][:])
            eng = nc.vector if j % 2 == 0 else nc.scalar
            if eng is nc.scalar:
                nc.scalar.activation(t_hT[kk][:], ptr[:], CPY)
            else:
                nc.vector.tensor_copy(t_hT[kk][:], ptr[:])
        if out_nat_t is not None:
            nc.sync.dma_start(out_nat_t[:, c*512:(c+1)*512], hb[:])
    if out_hT_t is not None:
        for k in range(KC):
            nc.sync.dma_start(out_hT_t[:, k*64:(k+1)*64], t_hT[k][:])


def build_v1(T=256, n_devices=1):
    nc = bacc.Bacc("TRN2", target_bir_lowering=False, debug=False, num_devices=n_devices)
    d = {}
    for l in range(6):
        d[f"whhT{l}"] = nc.dram_tensor(f"whhT{l}", [128, KC*3072], BF16, kind="ExternalInput")
        d[f"wihT{l}"] = nc.dram_tensor(f"wihT{l}", [128, KC*3072], BF16, kind="ExternalInput")
        d[f"brz{l}"] = nc.dram_tensor(f"brz{l}", [1, 2048], BF16, kind="ExternalInput")
        d[f"bhhn{l}"] = nc.dram_tensor(f"bhhn{l}", [1, 1024], BF16, kind="ExternalInput")
        d[f"bihn{l}"] = nc.dram_tensor(f"bihn{l}", [1, 1024], BF16, kind="ExternalInput")
    d["xT"] = nc.dram_tensor("xT", [T, 128, 512], BF16, kind="ExternalInput")
    d["eye"] = nc.dram_tensor("eye", [64, 64], BF16, kind="ExternalInput")
    tapes = [d["xT"]] + [nc.dram_tensor(f"tape{i}", [T, 128, 512], BF16) for i in (1, 2, 3, 4)]
    nat0 = nc.dram_tensor("nat0", [T, 64, 1024], BF16, kind="ExternalOutput")
    nat1 = nc.dram_tensor("nat1", [T, 64, 1024], BF16, kind="ExternalOutput")

    # chain l: (in_tape, out_tape, out_nat, state_line, relu)
    plan = [(0, 1, None, 0, False), (1, 2, None, 1, False),
            (2, 3, None, 0, True), (3, 4, None, 1, False),
            (4, None, nat0, 0, False), (4, None, nat1, 1, False)]

    with tile.TileContext(nc) as tc:
        with (
            tc.tile_pool(name="wpool", bufs=1) as wpool,
            tc.tile_pool(name="const", bufs=1) as const,
            tc.tile_pool(name="sbuf", bufs=3) as sbuf,
            tc.tile_pool(name="xring", bufs=6) as xring,
            tc.tile_pool(name="state", bufs=1) as state,
            tc.tile_pool(name="psum_rz", bufs=3, space="PSUM") as psum_rz,
            tc.tile_pool(name="psum_n", bufs=3, space="PSUM") as psum_n,
            tc.tile_pool(name="psum_tr", bufs=2, space="PSUM") as psum_tr,
        ):
            t_ones = const.tile([1, 64], BF16)
            nc.gpsimd.memset(t_ones[:], 1.0)
            t_eye = const.tile([64, 64], BF16, tag="eye", name="eye")
            nc.sync.dma_start(t_eye[:], d["eye"][:])
            t_whhT = wpool.tile([128, KC*3072], BF16, tag="whhT", name="whhT")
            t_wihT = wpool.tile([128, KC*3072], BF16, tag="wihT", name="wihT")
            t_brz = wpool.tile([1, 2048], BF16, tag="brz", name="brz")
            t_bhhn = wpool.tile([1, 1024], BF16, tag="bhhn", name="bhhn")
            t_bihn = wpool.tile([1, 1024], BF16, tag="bihn", name="bihn")
            # two state lines
            lines = []
            for s in range(2):
                hT = []
                for k in range(KC):
                    tk = state.tile([128, 64], BF16, tag=f"hT{s}_{k}", name=f"hT{s}_{k}")
                    nc.vector.memset(tk[:], 0.0)
                    hT.append(tk)
                hf = []
                for c in range(2):
                    tf = state.tile([64, 512], F32, tag=f"hf{s}_{c}", name=f"hf{s}_{c}")
                    nc.vector.memset(tf[:], 0.0)
                    hf.append(tf)
                lines.append((hT, hf))
            W = dict(brz=t_brz, bhhn=t_bhhn, bihn=t_bihn, whhT=t_whhT, wihT=t_wihT,
                     ones=t_ones, eye=t_eye)
            for l, (tin, tout, nat, sline, relu) in enumerate(plan):
                # (re)load weights for this chain
                nc.sync.dma_start(t_whhT[:], d[f"whhT{l}"][:])
                nc.sync.dma_start(t_wihT[:], d[f"wihT{l}"][:])
                nc.sync.dma_start(t_brz[:], d[f"brz{l}"][:])
                nc.sync.dma_start(t_bhhn[:], d[f"bhhn{l}"][:])
                nc.sync.dma_start(t_bihn[:], d[f"bihn{l}"][:])
                t_hT, t_hf = lines[sline]
                for t in range(T):
                    xt = xring.tile([128, 512], BF16, tag="xt", name="xt")
                    nc.sync.dma_start(xt[:], tapes[tin][t])
                    if relu:
                        nc.vector.tensor_scalar_max(xt[:], xt[:], 0.0)
                    emit_step(nc, sbuf, psum_rz, psum_n, psum_tr, W, xt[:], t_hT, t_hf,
                              out_nat_t=(nat[t] if nat is not None else None),
                              out_hT_t=(tapes[tout][t] if tout is not None else None))
    nc.compile()
    return nc, d


class BassRunner:
    def __init__(self, nc, n_cores):
        install_neuronx_cc_hook()
        self.nc = nc
        self.n_cores = n_cores
        partition_name = nc.partition_id_tensor.name if nc.partition_id_tensor else None
        in_names, out_names, out_avals, zero_outs = [], [], [], []
        for alloc in nc.m.functions[0].allocations:
            if not isinstance(alloc, mybir.MemoryLocationSet):
                continue
            name = alloc.memorylocations[0].name
            if alloc.kind == "ExternalInput":
                if name != partition_name:
                    in_names.append(name)
            elif alloc.kind == "ExternalOutput":
                shape = tuple(alloc.tensor_shape)
                dtype = mybir.dt.np(alloc.dtype)
                out_names.append(name)
                out_avals.append(jax.core.ShapedArray(shape, dtype))
                zero_outs.append(np.zeros(shape, dtype))
        self.in_names, self.out_names = in_names, out_names
        self.out_avals, self.zero_outs = out_avals, zero_outs
        n_params = len(in_names)
        n_outs = len(out_avals)
        all_in_names = list(in_names) + list(out_names)
        if partition_name is not None:
            all_in_names.append(partition_name)

        def _body(*args):
            operands = list(args)
            if partition_name is not None:
                operands.append(partition_id_tensor())
            outs = _bass_exec_p.bind(
                *operands,
                out_avals=tuple(out_avals),
                in_names=tuple(all_in_names),
                out_names=tuple(out_names),
                lowering_input_output_aliases=(),
                sim_require_finite=True,
                sim_require_nnan=True,
                nc=nc,
            )
            return tuple(outs)

        devices = jax.devices()[:n_cores]
        self.mesh = Mesh(np.asarray(devices), ("core",))
        in_specs = (PartitionSpec("core"),) * (n_params + n_outs)
        out_specs = (PartitionSpec("core"),) * n_outs
        # no donation: lets us call repeatedly with the same device arrays
        self.fn = jax.jit(shard_map(_body, mesh=self.mesh, in_specs=in_specs,
                                    out_specs=out_specs, check_rep=False),
                          keep_unused=True)

    def stage(self, in_maps):
        """Concat per-core inputs and place on devices; returns list of jax arrays."""
        n = self.n_cores
        concat = []
        for name in self.in_names:
            concat.append(np.concatenate([np.asarray(in_maps[c][name]) for c in range(n)], axis=0))
        for z in self.zero_outs:
            concat.append(np.zeros((n * z.shape[0], *z.shape[1:]), z.dtype))
        return [jax.device_put(a) for a in concat]

    def run(self, staged):
        outs = self.fn(*staged)
        jax.block_until_ready(outs)
        return outs

    def run_np(self, in_maps):
        outs = self.run(self.stage(in_maps))
        res = []
        for c in range(self.n_cores):
            m = {}
            for i, name in enumerate(self.out_names):
                m[name] = np.asarray(outs[i]).reshape(self.n_cores, *self.out_avals[i].shape)[c]
            res.append(m)
        return res

    def bench(self, in_maps, iters=5):
        staged = self.stage(in_maps)
        self.run(staged)  # warm/compile
        times = []
        for _ in range(iters):
            t0 = time.perf_counter()
            self.run(staged)
            times.append(time.perf_counter() - t0)
        return min(times), times

    def bench_pipelined(self, in_maps, iters=8):
        staged = self.stage(in_maps)
        self.run(staged)  # warm
        import time as _t, jax as _j
        t0 = _t.perf_counter()
        outs = None
        for _ in range(iters):
            outs = self.fn(*staged)
        _j.block_until_ready(outs)
        total = _t.perf_counter() - t0
        return total / iters


BF = ml_dtypes.bfloat16
T, B, H, V, L = 256, 64, 1024, 32000, 2
KC = 8

_cache = {}


def _pack_wT(W):
    Wr = np.asarray(W, np.float32).reshape(3*H, KC, 128)
    return np.ascontiguousarray(Wr.transpose(2, 1, 0).reshape(128, KC*3*H)).astype(BF)


def _pack_xT(x):
    # x [T, B, H] -> [T, 128, 512] bf16: xT[t, p, k*64 + b] = x[t, b, k*128+p]
    xr = np.asarray(x).reshape(T, B, KC, 128)
    return np.ascontiguousarray(xr.transpose(0, 3, 2, 1).reshape(T, 128, KC*B)).astype(BF)


def _chain_weights(l, Wih, Whh, bih, bhh):
    bih = np.asarray(bih, np.float32)
    bhh = np.asarray(bhh, np.float32)
    return {
        f"whhT{l}": _pack_wT(Whh), f"wihT{l}": _pack_wT(Wih),
        f"brz{l}": (bih+bhh)[None, :2048].astype(BF),
        f"bhhn{l}": bhh[None, 2048:].astype(BF),
        f"bihn{l}": bih[None, 2048:].astype(BF),
    }


def _get_runner():
    if "runner" not in _cache:
        nc, d = build_v1(T=T, n_devices=1)
        _cache["runner"] = BassRunner(nc, 1)
    return _cache["runner"]


def kernel(input, emb, enc_Wih, enc_Whh, enc_bih, enc_bhh,
           gru_Wih, gru_Whh, gru_bih, gru_bhh,
           cell_Wih, cell_Whh, cell_bih, cell_bhh,
           score_W, score_b):
    input = np.asarray(input)
    emb = np.asarray(emb, np.float32)
    tok = np.maximum(input, 0).astype(np.int64)
    x = np.maximum(emb[tok], 0.0)          # [T, B, H] relu(embedding)

    ins = {"xT": _pack_xT(x), "eye": np.eye(64, dtype=BF)}
    packs = [
        (enc_Wih[0], enc_Whh[0], enc_bih[0], enc_bhh[0]),
        (enc_Wih[1], enc_Whh[1], enc_bih[1], enc_bhh[1]),
        (gru_Wih[0], gru_Whh[0], gru_bih[0], gru_bhh[0]),
        (gru_Wih[1], gru_Whh[1], gru_bih[1], gru_bhh[1]),
        (cell_Wih[0], cell_Whh[0], cell_bih[0], cell_bhh[0]),
        (cell_Wih[1], cell_Whh[1], cell_bih[1], cell_bhh[1]),
    ]
    for l, p in enumerate(packs):
        ins.update(_chain_weights(l, *p))

    r = _get_runner()
    _cache["last_ins"] = ins
    res = r.run_np([ins])
    out0 = np.asarray(res[0]["nat0"], np.float32)   # [T, 64, 1024] cell0 outputs
    out1 = np.asarray(res[0]["nat1"], np.float32)

    w = np.asarray(score_W, np.float32)[0]          # [2048]
    bsc = float(np.asarray(score_b, np.float32)[0])
    logits = out0 @ w[:1024] + out1 @ w[1024:] + bsc
    return (1.0 / (1.0 + np.exp(-logits))).astype(np.float32)

